# revision 13
# baseline (speedup 1.0000x reference)
"""TRN2 Bass kernel for nn_EquivariantDiffusionModel (EGNN, B=64,N=29,E=812,L=9,H=256).

Sharding: pure data parallel, 8 molecules per NeuronCore (8 cores).

All feature tensors use transposed [feature, node/edge] layouts. The canonical
fully-connected graph (identical across the batch, as setup_inputs builds it)
lets gathers/scatters become structured ops after re-ordering edges to
e' = m*29 + i with j = (i+1+m) % 29:
  - i-gather: stride-0 broadcast read on DVE
  - j-gather: PE matmul against a constant 0/1 matrix RJ [29, 812]
  - scatter-add: segment reduction over the m axis
Everything is fp32: bf16 at any single site costs 5-10% end-to-end error
(the model's dynamics amplify rounding ~40x; fp32 lands at ~2e-5). PE matmuls
are issued as float32r (fp32 bits) which streams at full PE speed for moving
dims >= 256. Weights are streamed from HBM per layer and double-buffered.
Per-edge scalar math (sqrt via Newton rsqrt with an integer bit-trick seed --
no ACT table switches -- reciprocal, tanh gates) runs in a packed [128, 51]
"esc" layout: edge e of molecule mol sits at (p = 16*mol + q, f), e = q*51+f.

Non-canonical inputs (different edge_indices, non-one masks, nonzero biases)
fall back to an exact numpy implementation.
"""

import os
import sys

import numpy as np

for _p in ("/opt/trn_rl_repo", "/root/.axon_site/_ro/trn_rl_repo"):
    if os.path.isdir(_p) and _p not in sys.path:
        sys.path.insert(0, _p)

B, N, E, L, H, FA = 64, 29, 812, 9, 256, 5
SCALE = 15.0
NM = 8                 # molecules per core
NC = 8                 # cores
NT = NM * N            # 232
NTP = 256              # padded node free dim (f32r wants moving >= 256)
M28 = N - 1
ESC_F = 51             # esc packing: 812 = 15*51 + 47
ESC_Q = 15
ESC_R = 47
MAGIC = 0x5F3759DF

_BUILD_CACHE = {}
_LAST_EXEC_NS = {}
_LAST_RES = {}


def _f32r_sites():
    # f32r (11-bit mantissa matmul reads) measured on-device: the full
    # allowlist bought only ~10% end-to-end time while inflating rel err
    # from 1.1e-05 to 3.5e-02 (the model amplifies per-op rounding ~300x).
    # Keep everything exact fp32; the speed comes from engine scheduling.
    return frozenset()


F32R_SITES = _f32r_sites()


def _perm_ours_from_ref():
    """perm[e'] = reference edge index r for our e' = m*29+i, j=(i+1+m)%29."""
    perm = np.zeros(E, dtype=np.int64)
    for m in range(M28):
        for i in range(N):
            j = (i + 1 + m) % N
            mm = j if j < i else j - 1
            perm[m * N + i] = i * (N - 1) + mm
    return perm


def _canonical_edge_indices():
    return np.array(
        [(i, j) for i in range(N) for j in range(N) if i != j], dtype=np.int32
    )


def _rj_matrix():
    R = np.zeros((N, E), dtype=np.float32)
    for m in range(M28):
        for i in range(N):
            R[(i + 1 + m) % N, m * N + i] = 1.0
    return R


# --------------------------------------------------------------------------
# exact numpy fallback (for non-canonical inputs)
# --------------------------------------------------------------------------
def _numpy_forward(x_in, h_in, t, node_mask, edge_mask, edge_indices, p):
    def scatter_add(vals, idx):
        out = np.zeros((vals.shape[0], N, vals.shape[-1]), vals.dtype)
        for b in range(vals.shape[0]):
            np.add.at(out[b], idx[b], vals[b])
        return out

    def silu(v):
        return v * (1.0 / (1.0 + np.exp(-v)))

    def sig(v):
        return 1.0 / (1.0 + np.exp(-v))

    bidx = np.arange(x_in.shape[0])[:, None]
    idx_i, idx_j = edge_indices[..., 0], edge_indices[..., 1]
    h = (np.concatenate([h_in, t], -1) @ p["win_w"] + p["win_b"]).astype(np.float32)
    x = x_in.astype(np.float32)
    xi0, xj0 = x[bidx, idx_i], x[bidx, idx_j]
    a = np.sqrt(np.sum((xi0 - xj0) ** 2, -1, keepdims=True)) * edge_mask
    for l in range(L):
        x_i, x_j = x[bidx, idx_i], x[bidx, idx_j]
        diff = (x_i - x_j) * edge_mask
        d = np.sqrt(np.sum(diff**2, -1, keepdims=True))
        h_i, h_j = h[bidx, idx_i], h[bidx, idx_j]
        feat = np.concatenate([h_i, h_j, d**2, a], -1)
        u = silu(feat @ p["x_w1"][l] + p["x_b1"][l])
        u = silu(u @ p["x_w2"][l] + p["x_b2"][l])
        u = np.tanh(u @ p["x_w3"][l]) * SCALE
        u = u * diff / (d + 1.0)
        x = (x + scatter_add(u, idx_i)) * node_mask
        m = silu(feat @ p["e_w1"][l] + p["e_b1"][l])
        m = silu(m @ p["e_w2"][l] + p["e_b2"][l])
        e = sig(m @ p["att_w"][l] + p["att_b"][l])
        agg = scatter_add(e * m, idx_i)
        hm = silu(np.concatenate([h, agg], -1) @ p["h_w1"][l] + p["h_b1"][l])
        hm = hm @ p["h_w2"][l] + p["h_b2"][l]
        h = (h + hm) * node_mask
    xo = (x - x_in) * node_mask
    n = np.sum(node_mask, 1, keepdims=True)
    xo = (xo - np.sum(xo, 1, keepdims=True) / n) * node_mask
    ho = (h @ p["wout_w"] + p["wout_b"]) * node_mask
    return np.concatenate([xo, ho[..., :-1]], -1).astype(np.float32)


# --------------------------------------------------------------------------
# device kernel
# --------------------------------------------------------------------------
def _build(nlayers=L, dbg=False, f32r_sites=frozenset()):
    key = (nlayers, dbg, f32r_sites)
    if key in _BUILD_CACHE:
        return _BUILD_CACHE[key]

    from contextlib import ExitStack

    import concourse.bass as bass
    import concourse.tile as tile
    from concourse import bacc, mybir

    F32 = mybir.dt.float32
    F32R = mybir.dt.float32r
    I32 = mybir.dt.int32
    ALU = mybir.AluOpType
    ACTF = mybir.ActivationFunctionType
    ts = bass.ts

    nc = bacc.Bacc("TRN2", target_bir_lowering=False, debug=False, num_devices=NC)

    def din(name, shape):
        return nc.dram_tensor(name, list(shape), F32, kind="ExternalInput").ap()

    hT0_d = din("hT0", (FA + 1, NTP))
    x24_d = din("x24", (3 * NM, N))
    w1i_d = din("W1I", (L, 128, 2, 2, H))
    w1j_d = din("W1J", (L, 128, 2, 2, H))
    w2_d = din("W2", (L, 128, 2, 2, H))
    w38_d = din("W38", (L, 128, 2, NM, NM))
    attb_d = din("ATTB", (L, 128, 2, 128))
    hw1_d = din("HW1", (L, 128, 4, H))
    hw2_d = din("HW2", (L, 128, 2, H))
    wcd_d = din("WCD", (L, 2, 2, H))
    win_d = din("WIN", (FA + 1, NTP))
    wout_d = din("WOUT", (128, 2, FA + 1))
    rj_d = din("RJ", (N, E))
    obs_d = din("OBS", (3 * NM, NM))
    obw_d = din("OBW", (NM, 3 * NM))
    out_d = nc.dram_tensor("out", [FA + 3, NT], F32, kind="ExternalOutput").ap()

    def lab(ap, flag):
        # label-only f32r cast for DMA writers: DMAs move bytes, but the
        # BIR verifier requires every producer of an f32r-matmul operand
        # to declare an f32r output.
        return ap.bitcast(F32R) if flag else ap

    def rr(ap, site):
        # fp32 matmul = 4 cycles/col on PE; f32r (fp32 bits, 11-bit
        # mantissa read) = 1 cycle/col for moving>=256. The model is
        # chaotic (any per-op rounding amplifies ~300-500x into h), so
        # f32r is enabled only for sites measured insensitive in the
        # numpy rounding study; everything else stays exact fp32.
        return ap.bitcast(F32R) if site in f32r_sites else ap

    def vap(sliced, dims):
        # dims[0] is the partition dim with its step given in PARTITIONS;
        # bass APs use flat element addressing, so scale by the tensor's
        # partition pitch (taken from the sliced AP's own partition step).
        pitch = sliced.ap[0][0]
        d0 = [dims[0][0] * pitch, dims[0][1]]
        return bass.AP(
            tensor=sliced.tensor,
            offset=sliced.offset,
            ap=[d0] + [list(d) for d in dims[1:]],
        )

    def esc_scatter(dst_esc, src_row_of_mol, m):
        """DMA a [1, 812] molecule row into esc rows 16m..16m+16."""
        nc.sync.dma_start(
            vap(dst_esc[16 * m : 16 * m + ESC_Q, :], [[1, ESC_Q], [1, ESC_F]]),
            vap(src_row_of_mol[:, 0:765], [[1, 1], [ESC_F, ESC_Q], [1, ESC_F]]),
        )
        nc.sync.dma_start(
            vap(dst_esc[16 * m + ESC_Q : 16 * m + 16, 0:ESC_R], [[1, 1], [1, ESC_R]]),
            vap(src_row_of_mol[:, 765:E], [[1, 1], [1, ESC_R]]),
        )

    def esc_gather(dst_row_of_mol, src_esc, m):
        """DMA esc rows 16m..16m+16 back into a [1, 812] molecule row."""
        nc.sync.dma_start(
            vap(dst_row_of_mol[:, 0:765], [[1, 1], [ESC_F, ESC_Q], [1, ESC_F]]),
            vap(src_esc[16 * m : 16 * m + ESC_Q, :], [[1, ESC_Q], [1, ESC_F]]),
        )
        nc.sync.dma_start(
            vap(dst_row_of_mol[:, 765:E], [[1, 1], [1, ESC_R]]),
            vap(src_esc[16 * m + ESC_Q : 16 * m + 16, 0:ESC_R], [[1, 1], [1, ESC_R]]),
        )

    with tile.TileContext(nc) as tc, ExitStack() as ctx:
        pers = ctx.enter_context(tc.tile_pool(name="pers", bufs=1))
        wpool = ctx.enter_context(tc.tile_pool(name="wpool", bufs=2))
        st1 = ctx.enter_context(tc.tile_pool(name="st1", bufs=1))
        st2 = ctx.enter_context(tc.tile_pool(name="st2", bufs=2))
        geo = ctx.enter_context(tc.tile_pool(name="geo", bufs=1))
        mols = ctx.enter_context(tc.tile_pool(name="mols", bufs=2))
        pbig = ctx.enter_context(tc.tile_pool(name="pbig", bufs=2, space="PSUM"))
        psa = ctx.enter_context(tc.tile_pool(name="psa", bufs=2, space="PSUM"))
        ph8 = ctx.enter_context(tc.tile_pool(name="ph8", bufs=1, space="PSUM"))

        # ---- persistent constants / state ----
        any_pre1 = any(
            f"{l}:x1" in f32r_sites or f"{l}:e1" in f32r_sites for l in range(nlayers)
        )
        any_geos = any(f"geo_s:{l}" in f32r_sites for l in range(nlayers))
        # rjsa: rows 0..28 = RJ repeated per molecule block, rows 29..30 =
        # per-edge [s; a]. One K=31 matmul then computes the j-gather AND
        # the wcd @ [s; a] term of pre1 in a single instruction.
        rjsa = pers.tile([N + 2, E * NM], F32)
        for m in range(NM):
            nc.gpsimd.dma_start(
                lab(rjsa[0:N, m * E : (m + 1) * E], any_pre1),
                lab(rj_d[:], any_pre1),
            )
        obs_t = pers.tile([3 * NM, NM], F32)
        nc.gpsimd.dma_start(lab(obs_t[:], any_geos), lab(obs_d[:], any_geos))
        obw_t = pers.tile([NM, 3 * NM], F32)
        nc.gpsimd.dma_start(obw_t[:], obw_d[:])
        win_t = pers.tile([FA + 1, NTP], F32)
        nc.gpsimd.dma_start(win_t[:], win_d[:])
        wout_t = pers.tile([128, 2, FA + 1], F32)
        wr = "wout" in f32r_sites
        nc.gpsimd.dma_start(lab(wout_t[:], wr), lab(wout_d[:], wr))
        hT0_t = pers.tile([FA + 1, NTP], F32)
        nc.gpsimd.dma_start(hT0_t[:], hT0_d[:])
        x24i = pers.tile([3 * NM, N], F32)
        nc.gpsimd.dma_start(x24i[:], x24_d[:])
        ones1 = pers.tile([1, 128], F32)
        nc.vector.memset(ones1[:], 1.0)
        # per-partition constants for the Pool-engine Newton chains
        # (Pool has no tensor_scalar, so constants come in as stride-0
        # broadcast tensor operands): cols 0..3 = 1.5, 0.5, 2.0, 1.0
        cgeo = pers.tile([NM, 4], F32)
        for ci, cv in enumerate((1.5, 0.5, 2.0, 1.0)):
            nc.vector.memset(cgeo[:, ci : ci + 1], cv)

        def cb(ci):
            return vap(cgeo[:, ci : ci + 1], [[1, NM], [0, E]])

        sa2 = rjsa[N : N + 2, :]               # row0 = s (=d^2), row1 = a
        hf = pers.tile([128, 2, NTP], F32)     # h state fp32
        hb = pers.tile([128, 2, NTP], F32)     # matmul operand copy (padded)
        hbr = pers.tile([128, 2, NTP], F32)    # f32r-rounded operand copy

        def hb_for(site):
            return hbr if site in f32r_sites else hb
        x24 = pers.tile([3 * NM, N], F32)
        x24d = pers.tile([3 * NM, 2 * N], F32)

        # h0 = win_w.T @ [h_in; t]
        for mc in range(2):
            ps = psa.tile([128, NTP], F32, tag="psa")
            nc.tensor.matmul(
                ps[:], rr(win_t[:, ts(mc, 128)], "win"), rr(hT0_t[:], "win"),
                start=True, stop=True,
            )
            nc.vector.tensor_copy(hf[:, mc, :], ps[:])
            nc.vector.tensor_copy(hb[:, mc, :], ps[:])
            nc.vector.tensor_copy(hbr[:, mc, :].bitcast(F32R), ps[:])
        nc.vector.tensor_copy(x24[:], x24i[:])

        dbg_t = {}

        def dump(name, tile_ap):
            if not dbg:
                return
            if name not in dbg_t:
                dbg_t[name] = nc.dram_tensor(
                    "dbg_" + name, list(tile_ap.shape), F32, kind="ExternalOutput"
                ).ap()
            nc.sync.dma_start(dbg_t[name][:], tile_ap)

        for l in range(nlayers):
            # ---- stream layer weights (double-buffered) ----
            r_pre = f"{l}:x1" in f32r_sites or f"{l}:e1" in f32r_sites
            r_w2 = f"{l}:x2" in f32r_sites or f"{l}:e2" in f32r_sites
            w1i = wpool.tile([128, 2, 2, H], F32, tag="w1i")
            nc.gpsimd.dma_start(lab(w1i[:], r_pre), lab(w1i_d[l], r_pre))
            w1j = wpool.tile([128, 2, 2, H], F32, tag="w1j")
            nc.gpsimd.dma_start(lab(w1j[:], r_pre), lab(w1j_d[l], r_pre))
            w2 = wpool.tile([128, 2, 2, H], F32, tag="w2")
            nc.gpsimd.dma_start(lab(w2[:], r_w2), lab(w2_d[l], r_w2))
            w38 = wpool.tile([128, 2, NM, NM], F32, tag="w38")
            r_x3 = f"{l}:x3" in f32r_sites
            nc.gpsimd.dma_start(lab(w38[:], r_x3), lab(w38_d[l], r_x3))
            attb = wpool.tile([128, 2, 128], F32, tag="attb")
            r_att = f"{l}:att" in f32r_sites
            nc.gpsimd.dma_start(lab(attb[:], r_att), lab(attb_d[l], r_att))
            hw1 = wpool.tile([128, 4, H], F32, tag="hw1")
            r_h1 = f"{l}:h1" in f32r_sites
            nc.gpsimd.dma_start(lab(hw1[:], r_h1), lab(hw1_d[l], r_h1))
            hw2 = wpool.tile([128, 2, H], F32, tag="hw2")
            r_h2 = f"{l}:h2" in f32r_sites
            nc.gpsimd.dma_start(lab(hw2[:], r_h2), lab(hw2_d[l], r_h2))
            wcd = wpool.tile([2, 2, H], F32, tag="wcd")
            nc.gpsimd.dma_start(lab(wcd[:], r_pre), lab(wcd_d[l], r_pre))

            # ============ geometry part 1: diff / dsq (DVE) ============
            nc.vector.tensor_copy(x24d[:, 0:N], x24[:])
            nc.vector.tensor_copy(x24d[:, N : 2 * N], x24[:])
            diff = st2.tile([3 * NM, E], F32, tag="diff", bufs=1)
            nc.vector.tensor_tensor(
                vap(diff[:], [[1, 3 * NM], [N, M28], [1, N]]),
                vap(x24[:], [[1, 3 * NM], [0, M28], [1, N]]),
                vap(x24d[:, 1:], [[1, 3 * NM], [1, M28], [1, N]]),
                ALU.subtract,
            )
            if l == 0:
                dump("diff", diff[:])
            dsq = st2.tile([3 * NM, E], F32, tag="dsq", bufs=1)
            nc.vector.tensor_tensor(
                rr(dsq[:], f"geo_s:{l}"), diff[:], diff[:], ALU.mult
            )

            # ================= A-stage =================
            ai = st2.tile([128, 4, NTP], F32, tag="ai", bufs=1)
            for br in range(2):
                s1 = f"{l}:x1" if br == 0 else f"{l}:e1"
                for mc in range(2):
                    ps = psa.tile([128, NTP], F32, tag="psa")
                    for kc in range(2):
                        nc.tensor.matmul(
                            ps[:],
                            rr(w1i[:, br, kc, ts(mc, 128)], s1),
                            rr(hb_for(s1)[:, kc, :], s1),
                            start=(kc == 0),
                            stop=(kc == 1),
                        )
                    nc.vector.tensor_copy(ai[:, 2 * br + mc, :], ps[:])
            if l == 0:
                dump("ai", ai[:])
            # ajt rows 0..28: h_j @ w1j per node; rows 29..30: wcd, so one
            # K=31 matmul against rjsa computes j-gather + wcd @ [s; a]
            ajt = st1.tile([N + 2, 2, NM, H], F32, tag="ajt")
            for br in range(2):
                s1 = f"{l}:x1" if br == 0 else f"{l}:e1"
                # wcd -> ajt rows 29..30 for every molecule slot (label-only
                # f32r DMA; rounding happens at the matmul read)
                wsl = wcd[:, br, :]
                dsl = ajt[N : N + 2, br, 0, :]
                nc.gpsimd.dma_start(
                    lab(
                        bass.AP(
                            tensor=dsl.tensor,
                            offset=dsl.offset,
                            ap=[list(dsl.ap[0]), [H, NM], [1, H]],
                        ),
                        r_pre,
                    ),
                    lab(
                        bass.AP(
                            tensor=wsl.tensor,
                            offset=wsl.offset,
                            ap=[list(wsl.ap[0]), [0, NM], [1, H]],
                        ),
                        r_pre,
                    ),
                )
                for nk in range(2):
                    ps = psa.tile([116, NTP], F32, tag="psa")
                    for kc in range(2):
                        nc.tensor.matmul(
                            ps[:, 0:H],
                            rr(hb_for(s1)[:, kc, nk * 116 : nk * 116 + 116], s1),
                            rr(w1j[:, br, kc, :], s1),
                            start=(kc == 0),
                            stop=(kc == 1),
                        )
                    ajsb = st2.tile([116, H], F32, tag="ajsb")
                    nc.vector.tensor_copy(rr(ajsb[:], s1), ps[:, 0:H])
                    for mm in range(4):
                        nc.sync.dma_start(
                            lab(ajt[0:N, br, nk * 4 + mm, :], r_pre),
                            lab(ajsb[29 * mm : 29 * mm + 29, :], r_pre),
                        )

            # ========= geometry part 2: s = per-mol xyz sum (PE) =========
            s8p = ph8.tile([NM, 1024], F32, tag="ph8")
            gs = f"geo_s:{l}"
            nc.tensor.matmul(
                s8p[:, 0:510], rr(obs_t[:], gs), rr(dsq[:, 0:510], gs),
                start=True, stop=True,
            )
            nc.tensor.matmul(
                s8p[:, 512:814], rr(obs_t[:], gs), rr(dsq[:, 510:E], gs),
                start=True, stop=True,
            )
            s8sb = geo.tile([NM, E], F32, tag="s8sb")
            nc.vector.tensor_copy(s8sb[:, 0:510], s8p[:, 0:510])
            nc.vector.tensor_copy(s8sb[:, 510:E], s8p[:, 512:814])
            nc.sync.dma_start(
                lab(vap(sa2[0:1, :], [[1, 1], [E, NM], [1, E]]), any_pre1),
                lab(vap(s8sb[:], [[1, NM], [1, E]]), any_pre1),
            )
            if l == 0:
                dump("s8sb", s8sb[:])

            def emit_newton():
                # d = s * rsqrt(s) (bit-trick seed + 3 Newton iters), then
                # w = 1/(d+1) (fast-reciprocal seed + 3 Newton iters). The
                # serial float chain runs on the otherwise-idle Pool engine:
                # on the DVE it executes as one solid ~20us burst that
                # starves the psv-add -> silu chain feeding the PE. Pool has
                # no tensor_scalar, so the int seeds stay on the DVE (cheap)
                # and iteration constants come from stride-0 broadcasts.
                it8 = geo.tile([NM, E], I32, tag="it8")
                nc.vector.tensor_scalar(
                    it8[:], s8sb[:].bitcast(I32), 1, None,
                    ALU.logical_shift_right
                )
                nc.vector.tensor_scalar(it8[:], it8[:], -1, None,
                                        ALU.bitwise_xor)
                nc.vector.tensor_scalar(it8[:], it8[:], MAGIC + 1, None,
                                        ALU.add)
                r8 = it8[:].bitcast(F32)
                sh8 = geo.tile([NM, E], F32, tag="sh8")
                nc.gpsimd.tensor_tensor(sh8[:], s8sb[:], cb(1), ALU.mult)
                t8 = geo.tile([NM, E], F32, tag="t8")
                for _ in range(3):
                    nc.gpsimd.tensor_tensor(t8[:], r8, r8, ALU.mult)
                    nc.gpsimd.tensor_tensor(t8[:], t8[:], sh8[:], ALU.mult)
                    nc.gpsimd.tensor_tensor(t8[:], cb(0), t8[:],
                                            ALU.subtract)
                    nc.gpsimd.tensor_tensor(r8, r8, t8[:], ALU.mult)
                d8 = geo.tile([NM, E], F32, tag="d8")
                nc.gpsimd.tensor_tensor(d8[:], s8sb[:], r8, ALU.mult)
                if l == 0:
                    nc.sync.dma_start(
                        lab(vap(sa2[1:2, :], [[1, 1], [E, NM], [1, E]]),
                            any_pre1),
                        lab(vap(d8[:], [[1, NM], [1, E]]), any_pre1),
                    )
                # w = 1/(1+d): y0 = bits(0x7EF311C3 - bits(1+d)), then
                # y <- y*(2 - (1+d)*y) three times (quadratic convergence
                # from the ~5e-2 seed error down to fp32 exact).
                nc.gpsimd.tensor_tensor(t8[:], d8[:], cb(3), ALU.add)
                iw8 = geo.tile([NM, E], I32, tag="iw8")
                nc.vector.tensor_scalar(iw8[:], t8[:].bitcast(I32), -1, None,
                                        ALU.bitwise_xor)
                nc.vector.tensor_scalar(iw8[:], iw8[:], 0x7EF311C4, None,
                                        ALU.add)
                w8 = iw8[:].bitcast(F32)
                u8 = geo.tile([NM, E], F32, tag="u8")
                for _ in range(3):
                    nc.gpsimd.tensor_tensor(u8[:], t8[:], w8, ALU.mult)
                    nc.gpsimd.tensor_tensor(u8[:], cb(2), u8[:],
                                            ALU.subtract)
                    nc.gpsimd.tensor_tensor(w8, w8, u8[:], ALU.mult)
                if l == 0:
                    dump("sa2", sa2[:])
                    dump("w8", w8)
                return w8

            w8 = emit_newton()

            # ============ branch MLPs (e: br=1 first, then x: br=0) ============
            # Stage-major with lag interleave: the PE stream alternates
            # pre1[m] / w2[m-2] / head[m-3] so every matmul's inputs (silu
            # outputs) were produced while the PE ran other molecules.
            # The e-branch gating (em mult + segment reduce) runs on the
            # Pool engine so the DVE never head-of-line-blocks the silu
            # chain that feeds the PE.
            if l == 0:
                dump("ajt", ajt[:])
            agg = st2.tile([128, 2, NTP], F32, tag="agg", bufs=1)
            nc.vector.memset(agg[:, :, NT:NTP], 0.0)
            php = ph8.tile([NM, 1024], F32, tag="ph8")
            sil1_t = {}
            sil2_t = {}

            def pre1_op(br, m):
                s1 = f"{l}:x1" if br == 0 else f"{l}:e1"
                s2 = f"{l}:x2" if br == 0 else f"{l}:e2"
                sil1 = mols.tile([128, 2, E], F32, tag="sil1", bufs=3)
                sil1_t[(br, m)] = sil1
                for mc in range(2):
                    ps = pbig.tile([128, 1024], F32, tag="pp")
                    for col in range(2):
                        po = ps[:, col * 512 : col * 512 + 406]
                        c0 = m * E + col * 406
                        nc.tensor.matmul(
                            po,
                            rr(ajt[:, br, m, ts(mc, 128)], s1),
                            rr(rjsa[:, c0 : c0 + 406], s1),
                            start=True,
                            stop=True,
                        )
                    psv = vap(ps[:], [[1, 128], [512, 2], [1, 406]])
                    nc.vector.tensor_tensor(
                        psv,
                        vap(
                            ai[:, 2 * br + mc, m * N : (m + 1) * N],
                            [[1, 128], [0, M28], [1, N]],
                        ),
                        psv,
                        ALU.add,
                    )
                    nc.scalar.activation(
                        rr(vap(sil1[:, mc, :], [[1, 128], [406, 2], [1, 406]]),
                           s2),
                        psv,
                        ACTF.Silu,
                    )
                if l == 0 and m == 0:
                    dump(f"sil1_{br}", sil1[:])

            def w2_op(br, m):
                s2 = f"{l}:x2" if br == 0 else f"{l}:e2"
                shead = f"{l}:x3" if br == 0 else f"{l}:att"
                sil1 = sil1_t.pop((br, m))
                sil2 = mols.tile([128, 2, E], F32, tag="sil2", bufs=2)
                sil2_t[(br, m)] = sil2
                for mc in range(2):
                    ps = pbig.tile([128, 1024], F32, tag="pp")
                    for col in range(2):
                        po = ps[:, col * 512 : col * 512 + 406]
                        for kc in range(2):
                            nc.tensor.matmul(
                                po,
                                rr(w2[:, br, kc, ts(mc, 128)], s2),
                                rr(sil1[:, kc, col * 406 : col * 406 + 406], s2),
                                start=(kc == 0),
                                stop=(kc == 1),
                            )
                    nc.scalar.activation(
                        rr(vap(sil2[:, mc, :], [[1, 128], [406, 2], [1, 406]]),
                           shead),
                        vap(ps[:], [[1, 128], [512, 2], [1, 406]]),
                        ACTF.Silu,
                    )
                if l == 0 and m == 0:
                    dump(f"sil2_{br}", sil2[:])

            def head_op(br, m):
                sil2 = sil2_t.pop((br, m))
                if br == 0:
                    for col in range(2):
                        for kc in range(2):
                            nc.tensor.matmul(
                                php[:, col * 512 : col * 512 + 406],
                                rr(w38[:, kc, m, :], f"{l}:x3"),
                                rr(sil2[:, kc, col * 406 : col * 406 + 406],
                                   f"{l}:x3"),
                                start=(m == 0 and kc == 0),
                                stop=(m == NM - 1 and kc == 1),
                                skip_group_check=True,
                            )
                else:
                    atp = pbig.tile([128, 1024], F32, tag="pp")
                    for col in range(2):
                        for kc in range(2):
                            nc.tensor.matmul(
                                atp[:, col * 512 : col * 512 + 406],
                                rr(attb[:, kc, :], f"{l}:att"),
                                rr(sil2[:, kc, col * 406 : col * 406 + 406],
                                   f"{l}:att"),
                                start=(kc == 0),
                                stop=(kc == 1),
                            )
                    eg_sb = mols.tile([128, E], F32, tag="eg_sb")
                    nc.scalar.activation(
                        vap(eg_sb[:], [[1, 128], [406, 2], [1, 406]]),
                        vap(atp[:], [[1, 128], [512, 2], [1, 406]]),
                        ACTF.Tanh,
                        scale=0.5,
                    )
                    nc.vector.tensor_scalar(
                        eg_sb[:], eg_sb[:], 0.5, 0.5, ALU.mult, ALU.add
                    )
                    if l == 0 and m == 0:
                        dump("eg_sb", eg_sb[:])
                    em = mols.tile([128, 2, E], F32, tag="em", bufs=2)
                    for mc in range(2):
                        nc.gpsimd.tensor_tensor(
                            em[:, mc, :], sil2[:, mc, :], eg_sb[:], ALU.mult
                        )
                        # segment-sum of the 28 m'-blocks via an in-place
                        # contiguous add tree (Pool can't do free-axis
                        # reduce); 28*29 = 812 → 406 → 203 → 87 → 29
                        ev = em[:, mc, :]
                        for lo, mid, w in (
                            (0, 406, 406),   # 28 -> 14 blocks
                            (0, 203, 203),   # 14 -> 7
                            (0, 87, 87),     # 7 -> 3 (+1 leftover at 174)
                            (0, 29, 29),     # 3 -> 1 (+1 leftover at 58)
                            (0, 58, 29),
                        ):
                            nc.gpsimd.tensor_tensor(
                                ev[:, lo : lo + w], ev[:, lo : lo + w],
                                ev[:, mid : mid + w], ALU.add,
                            )
                        nc.gpsimd.tensor_tensor(
                            agg[:, mc, m * N : (m + 1) * N],
                            ev[:, 0:N], ev[:, 174 : 174 + N], ALU.add,
                        )

            for br in (1, 0):
                for step in range(NM + 3):
                    if step < NM:
                        pre1_op(br, step)
                    if 0 <= step - 2 < NM:
                        w2_op(br, step - 2)
                    if 0 <= step - 3 < NM:
                        head_op(br, step - 3)

            if True:
                if True:
                    # ---- x tail ----
                    phi8 = geo.tile([NM, E], F32, tag="phi8")
                    nc.vector.tensor_copy(
                        phi8[:],
                        vap(php[:], [[1, NM], [512, 2], [1, 406]]),
                    )
                    if l == 0:
                        dump("phi8", phi8[:])
                    g8 = geo.tile([NM, E], F32, tag="g8")
                    nc.scalar.activation(g8[:], phi8[:], ACTF.Tanh)
                    wg8 = geo.tile([NM, E], F32, tag="wg8")
                    nc.vector.tensor_tensor(wg8[:], w8, g8[:], ALU.mult)
                    wg24 = pbig.tile([3 * NM, 1024], F32, tag="pp")
                    gu = f"geo_u:{l}"
                    nc.tensor.matmul(
                        wg24[:, 0:406],
                        rr(obw_t[:], gu),
                        rr(wg8[:, 0:406], gu),
                        start=True,
                        stop=True,
                    )
                    nc.tensor.matmul(
                        wg24[:, 512:918],
                        rr(obw_t[:], gu),
                        rr(wg8[:, 406:E], gu),
                        start=True,
                        stop=True,
                    )
                    u_vec = st2.tile([3 * NM, E], F32, tag="u_vec", bufs=1)
                    nc.vector.tensor_tensor(
                        vap(u_vec[:], [[1, 3 * NM], [406, 2], [1, 406]]),
                        vap(diff[:], [[1, 3 * NM], [406, 2], [1, 406]]),
                        vap(wg24[:], [[1, 3 * NM], [512, 2], [1, 406]]),
                        ALU.mult,
                    )
                    if l == 0:
                        dump("wg8", wg8[:])
                        dump("u_vec", u_vec[:])
                    xinc = st2.tile([3 * NM, N], F32, tag="xinc")
                    nc.vector.tensor_reduce(
                        xinc[:],
                        vap(u_vec[:], [[1, 3 * NM], [1, N], [N, M28]]),
                        axis=mybir.AxisListType.X,
                        op=ALU.add,
                    )
                    x24n = pers.tile([3 * NM, N], F32, tag=f"x24n_{l % 2}")
                    nc.vector.tensor_tensor(x24n[:], x24[:], xinc[:], ALU.add)
                    x24 = x24n

            if l == 0:
                dump("agg", agg[:])
            # ================= h update =================
            hm1 = st2.tile([128, 2, NTP], F32, tag="hm1", bufs=1)
            rhs_list = [hb[:, 0, :], hb[:, 1, :], agg[:, 0, :], agg[:, 1, :]]
            for mc in range(2):
                ps = psa.tile([128, NTP], F32, tag="psa")
                for kc in range(4):
                    nc.tensor.matmul(
                        ps[:],
                        rr(hw1[:, kc, ts(mc, 128)], f"{l}:h1"),
                        rr(rhs_list[kc], f"{l}:h1"),
                        start=(kc == 0),
                        stop=(kc == 3),
                    )
                nc.scalar.activation(
                    rr(hm1[:, mc, :], f"{l}:h2"), ps[:], ACTF.Silu
                )
            hfn = pers.tile([128, 2, NTP], F32, tag=f"hf_{l % 2}")
            for mc in range(2):
                ps = psa.tile([128, NTP], F32, tag="psa")
                for kc in range(2):
                    nc.tensor.matmul(
                        ps[:],
                        rr(hw2[:, kc, ts(mc, 128)], f"{l}:h2"),
                        rr(hm1[:, kc, :], f"{l}:h2"),
                        start=(kc == 0),
                        stop=(kc == 1),
                    )
                nc.vector.tensor_tensor(hfn[:, mc, :], hf[:, mc, :], ps[:], ALU.add)
                nc.vector.tensor_copy(hb[:, mc, :], hfn[:, mc, :])
                nc.vector.tensor_copy(
                    hbr[:, mc, :].bitcast(F32R), hfn[:, mc, :]
                )
            hf = hfn
            if l == 0:
                dump("hf1", hf[:])
                dump("x24_1", x24[:])

        # ================= output =================
        xd = st2.tile([3 * NM, N], F32, tag="xd")
        nc.vector.tensor_tensor(xd[:], x24[:], x24i[:], ALU.subtract)
        mean = st2.tile([3 * NM, 1], F32, tag="mean")
        nc.vector.tensor_reduce(
            mean[:], xd[:], axis=mybir.AxisListType.X, op=ALU.add
        )
        nc.vector.tensor_scalar(mean[:], mean[:], 1.0 / N, None, ALU.mult)
        xo = st2.tile([3 * NM, N], F32, tag="xo")
        nc.vector.tensor_scalar(xo[:], xd[:], mean[:], None, ALU.subtract)
        out8 = st1.tile([FA + 3, NT], F32, tag="out8")
        for m in range(NM):
            nc.sync.dma_start(
                out8[0:3, m * N : (m + 1) * N], xo[3 * m : 3 * m + 3, :]
            )
        ps = psa.tile([FA + 1, NTP], F32, tag="psa")
        for kc in range(2):
            nc.tensor.matmul(
                ps[:],
                rr(wout_t[:, kc, :], "wout"),
                rr(hb_for("wout")[:, kc, :], "wout"),
                start=(kc == 0),
                stop=(kc == 1),
            )
        hosb = st2.tile([FA + 1, NTP], F32, tag="hosb")
        nc.vector.tensor_copy(hosb[:], ps[:])
        nc.sync.dma_start(out8[3 : 3 + FA, :], hosb[0:FA, 0:NT])
        nc.sync.dma_start(out_d[:], out8[:])

    nc.compile()
    _BUILD_CACHE[key] = nc
    return nc


# --------------------------------------------------------------------------
# host side
# --------------------------------------------------------------------------
def _is_canonical(inputs):
    ei = np.asarray(inputs["edge_indices"])
    if not np.all(ei == _canonical_edge_indices()[None]):
        return False
    if not np.all(np.asarray(inputs["node_mask"]) == 1.0):
        return False
    if not np.all(np.asarray(inputs["edge_mask"]) == 1.0):
        return False
    for k in ("win_b", "wout_b", "e_b1", "e_b2", "att_b", "h_b1", "h_b2",
              "x_b1", "x_b2"):
        if not np.all(np.asarray(inputs[k]) == 0.0):
            return False
    return True


def _prep_shared(inputs):
    f = np.float32
    x_w1 = np.asarray(inputs["x_w1"], f)
    e_w1 = np.asarray(inputs["e_w1"], f)
    x_w2 = np.asarray(inputs["x_w2"], f)
    e_w2 = np.asarray(inputs["e_w2"], f)
    x_w3 = np.asarray(inputs["x_w3"], f)
    att_w = np.asarray(inputs["att_w"], f)
    h_w1 = np.asarray(inputs["h_w1"], f)
    h_w2 = np.asarray(inputs["h_w2"], f)

    W1I = np.zeros((L, 128, 2, 2, H), f)
    W1J = np.zeros((L, 128, 2, 2, H), f)
    W2 = np.zeros((L, 128, 2, 2, H), f)
    W38 = np.zeros((L, 128, 2, NM, NM), f)
    ATTB = np.zeros((L, 128, 2, 128), f)
    HW1 = np.zeros((L, 128, 4, H), f)
    HW2 = np.zeros((L, 128, 2, H), f)
    WCD = np.zeros((L, 2, 2, H), f)
    for l in range(L):
        for br, w1 in enumerate((x_w1[l], e_w1[l])):
            for kc in range(2):
                W1I[l, :, br, kc, :] = w1[kc * 128 : (kc + 1) * 128, :]
                W1J[l, :, br, kc, :] = w1[H + kc * 128 : H + (kc + 1) * 128, :]
            WCD[l, 0, br, :] = w1[2 * H, :]
            WCD[l, 1, br, :] = w1[2 * H + 1, :]
        for br, w2 in enumerate((x_w2[l], e_w2[l])):
            for kc in range(2):
                W2[l, :, br, kc, :] = w2[kc * 128 : (kc + 1) * 128, :]
        for kc in range(2):
            for m in range(NM):
                W38[l, :, kc, m, m] = x_w3[l][kc * 128 : (kc + 1) * 128, 0]
            ATTB[l, :, kc, :] = att_w[l][kc * 128 : (kc + 1) * 128, 0:1]
            HW2[l, :, kc, :] = h_w2[l][kc * 128 : (kc + 1) * 128, :]
        for kc in range(4):
            HW1[l, :, kc, :] = h_w1[l][kc * 128 : (kc + 1) * 128, :]

    WIN = np.zeros((FA + 1, NTP), f)
    WIN[:, :H] = np.asarray(inputs["win_w"], f)
    WOUT = np.zeros((128, 2, FA + 1), f)
    wout = np.asarray(inputs["wout_w"], f)
    for kc in range(2):
        WOUT[:, kc, :] = wout[kc * 128 : (kc + 1) * 128, :]
    RJ = _rj_matrix()
    OBS = np.kron(np.eye(NM, dtype=f), np.ones((3, 1), f))
    OBW = (SCALE * np.kron(np.eye(NM, dtype=f), np.ones((1, 3), f))).astype(f)
    return dict(
        W1I=W1I, W1J=W1J, W2=W2, W38=W38, ATTB=ATTB, HW1=HW1, HW2=HW2,
        WCD=WCD, WIN=WIN, WOUT=WOUT, RJ=RJ, OBS=OBS, OBW=OBW,
    )


def _prep_core(inputs, c):
    f = np.float32
    sl = slice(c * NM, (c + 1) * NM)
    h_in = np.asarray(inputs["h_in"], f)[sl]
    t = np.asarray(inputs["t"], f)[sl]
    x_in = np.asarray(inputs["x_in"], f)[sl]
    hT0 = np.zeros((FA + 1, NTP), f)
    hT0[:, :NT] = (
        np.concatenate([h_in, t], -1).reshape(NT, FA + 1).T
    )
    x24 = np.ascontiguousarray(x_in.transpose(0, 2, 1).reshape(3 * NM, N))
    return {"hT0": hT0, "x24": x24}


def kernel(**inputs):
    if not _is_canonical(inputs):
        p = {k: np.asarray(v, np.float32) for k, v in inputs.items()
             if k not in ("edge_indices",)}
        return _numpy_forward(
            np.asarray(inputs["x_in"], np.float32),
            np.asarray(inputs["h_in"], np.float32),
            np.asarray(inputs["t"], np.float32),
            np.asarray(inputs["node_mask"], np.float32),
            np.asarray(inputs["edge_mask"], np.float32),
            np.asarray(inputs["edge_indices"]),
            p,
        )

    from concourse.bass_utils import run_bass_kernel_spmd

    nc = _build(f32r_sites=F32R_SITES)
    _LAST_EXEC_NS.clear()
    shared = _prep_shared(inputs)
    in_maps = []
    for c in range(NC):
        m = dict(shared)
        m.update(_prep_core(inputs, c))
        in_maps.append(m)
    res = run_bass_kernel_spmd(nc, in_maps, core_ids=list(range(NC)))
    _LAST_RES["res"] = res
    if res.exec_time_ns is not None:
        _LAST_EXEC_NS["ns"] = res.exec_time_ns
    out = np.zeros((B, N, FA + 3), np.float32)
    for c in range(NC):
        o = res.results[c]["out"]  # [8, 232]
        out[c * NM : (c + 1) * NM] = o.reshape(FA + 3, NM, N).transpose(1, 2, 0)
    return out



# revision 14
# speedup vs baseline: 1.0505x; 1.0505x over previous
"""TRN2 Bass kernel for nn_EquivariantDiffusionModel (EGNN, B=64,N=29,E=812,L=9,H=256).

Sharding: pure data parallel, 8 molecules per NeuronCore (8 cores).

All feature tensors use transposed [feature, node/edge] layouts. The canonical
fully-connected graph (identical across the batch, as setup_inputs builds it)
lets gathers/scatters become structured ops after re-ordering edges to
e' = m*29 + i with j = (i+1+m) % 29:
  - i-gather: stride-0 broadcast read on DVE
  - j-gather: PE matmul against a constant 0/1 matrix RJ [29, 812]
  - scatter-add: segment reduction over the m axis
Everything is fp32: bf16 at any single site costs 5-10% end-to-end error
(the model's dynamics amplify rounding ~40x; fp32 lands at ~2e-5). PE matmuls
are issued as float32r (fp32 bits) which streams at full PE speed for moving
dims >= 256. Weights are streamed from HBM per layer and double-buffered.
Per-edge scalar math (sqrt via Newton rsqrt with an integer bit-trick seed --
no ACT table switches -- reciprocal, tanh gates) runs in a packed [128, 51]
"esc" layout: edge e of molecule mol sits at (p = 16*mol + q, f), e = q*51+f.

Non-canonical inputs (different edge_indices, non-one masks, nonzero biases)
fall back to an exact numpy implementation.
"""

import os
import sys

import numpy as np

for _p in ("/opt/trn_rl_repo", "/root/.axon_site/_ro/trn_rl_repo"):
    if os.path.isdir(_p) and _p not in sys.path:
        sys.path.insert(0, _p)

B, N, E, L, H, FA = 64, 29, 812, 9, 256, 5
SCALE = 15.0
NM = 8                 # molecules per core
NC = 8                 # cores
NT = NM * N            # 232
NTP = 256              # padded node free dim (f32r wants moving >= 256)
M28 = N - 1
ESC_F = 51             # esc packing: 812 = 15*51 + 47
ESC_Q = 15
ESC_R = 47
MAGIC = 0x5F3759DF

_BUILD_CACHE = {}
_LAST_EXEC_NS = {}
_LAST_RES = {}


def _f32r_sites():
    # f32r (11-bit mantissa matmul reads) measured on-device: the full
    # allowlist bought only ~10% end-to-end time while inflating rel err
    # from 1.1e-05 to 3.5e-02 (the model amplifies per-op rounding ~300x).
    # Keep everything exact fp32; the speed comes from engine scheduling.
    return frozenset()


F32R_SITES = _f32r_sites()


def _perm_ours_from_ref():
    """perm[e'] = reference edge index r for our e' = m*29+i, j=(i+1+m)%29."""
    perm = np.zeros(E, dtype=np.int64)
    for m in range(M28):
        for i in range(N):
            j = (i + 1 + m) % N
            mm = j if j < i else j - 1
            perm[m * N + i] = i * (N - 1) + mm
    return perm


def _canonical_edge_indices():
    return np.array(
        [(i, j) for i in range(N) for j in range(N) if i != j], dtype=np.int32
    )


def _rj_matrix():
    R = np.zeros((N, E), dtype=np.float32)
    for m in range(M28):
        for i in range(N):
            R[(i + 1 + m) % N, m * N + i] = 1.0
    return R


# --------------------------------------------------------------------------
# exact numpy fallback (for non-canonical inputs)
# --------------------------------------------------------------------------
def _numpy_forward(x_in, h_in, t, node_mask, edge_mask, edge_indices, p):
    def scatter_add(vals, idx):
        out = np.zeros((vals.shape[0], N, vals.shape[-1]), vals.dtype)
        for b in range(vals.shape[0]):
            np.add.at(out[b], idx[b], vals[b])
        return out

    def silu(v):
        return v * (1.0 / (1.0 + np.exp(-v)))

    def sig(v):
        return 1.0 / (1.0 + np.exp(-v))

    bidx = np.arange(x_in.shape[0])[:, None]
    idx_i, idx_j = edge_indices[..., 0], edge_indices[..., 1]
    h = (np.concatenate([h_in, t], -1) @ p["win_w"] + p["win_b"]).astype(np.float32)
    x = x_in.astype(np.float32)
    xi0, xj0 = x[bidx, idx_i], x[bidx, idx_j]
    a = np.sqrt(np.sum((xi0 - xj0) ** 2, -1, keepdims=True)) * edge_mask
    for l in range(L):
        x_i, x_j = x[bidx, idx_i], x[bidx, idx_j]
        diff = (x_i - x_j) * edge_mask
        d = np.sqrt(np.sum(diff**2, -1, keepdims=True))
        h_i, h_j = h[bidx, idx_i], h[bidx, idx_j]
        feat = np.concatenate([h_i, h_j, d**2, a], -1)
        u = silu(feat @ p["x_w1"][l] + p["x_b1"][l])
        u = silu(u @ p["x_w2"][l] + p["x_b2"][l])
        u = np.tanh(u @ p["x_w3"][l]) * SCALE
        u = u * diff / (d + 1.0)
        x = (x + scatter_add(u, idx_i)) * node_mask
        m = silu(feat @ p["e_w1"][l] + p["e_b1"][l])
        m = silu(m @ p["e_w2"][l] + p["e_b2"][l])
        e = sig(m @ p["att_w"][l] + p["att_b"][l])
        agg = scatter_add(e * m, idx_i)
        hm = silu(np.concatenate([h, agg], -1) @ p["h_w1"][l] + p["h_b1"][l])
        hm = hm @ p["h_w2"][l] + p["h_b2"][l]
        h = (h + hm) * node_mask
    xo = (x - x_in) * node_mask
    n = np.sum(node_mask, 1, keepdims=True)
    xo = (xo - np.sum(xo, 1, keepdims=True) / n) * node_mask
    ho = (h @ p["wout_w"] + p["wout_b"]) * node_mask
    return np.concatenate([xo, ho[..., :-1]], -1).astype(np.float32)


# --------------------------------------------------------------------------
# device kernel
# --------------------------------------------------------------------------
def _build(nlayers=L, dbg=False, f32r_sites=frozenset()):
    key = (nlayers, dbg, f32r_sites)
    if key in _BUILD_CACHE:
        return _BUILD_CACHE[key]

    from contextlib import ExitStack

    import concourse.bass as bass
    import concourse.tile as tile
    from concourse import bacc, mybir

    F32 = mybir.dt.float32
    F32R = mybir.dt.float32r
    I32 = mybir.dt.int32
    ALU = mybir.AluOpType
    ACTF = mybir.ActivationFunctionType
    ts = bass.ts

    nc = bacc.Bacc("TRN2", target_bir_lowering=False, debug=False, num_devices=NC)

    def din(name, shape):
        return nc.dram_tensor(name, list(shape), F32, kind="ExternalInput").ap()

    hT0_d = din("hT0", (FA + 1, NTP))
    x24_d = din("x24", (3 * NM, N))
    w1i_d = din("W1I", (L, 128, 2, 2, H))
    w1j_d = din("W1J", (L, 128, 2, 2, H))
    w2_d = din("W2", (L, 128, 2, 2, H))
    w38_d = din("W38", (L, 128, 2, NM, NM))
    attb_d = din("ATTB", (L, 128, 2, 128))
    hw1_d = din("HW1", (L, 128, 4, H))
    hw2_d = din("HW2", (L, 128, 2, H))
    wcd_d = din("WCD", (L, 2, 2, H))
    win_d = din("WIN", (FA + 1, NTP))
    wout_d = din("WOUT", (128, 2, FA + 1))
    rj_d = din("RJ", (N, E))
    obs_d = din("OBS", (3 * NM, NM))
    obw_d = din("OBW", (NM, 3 * NM))
    out_d = nc.dram_tensor("out", [FA + 3, NT], F32, kind="ExternalOutput").ap()

    def lab(ap, flag):
        # label-only f32r cast for DMA writers: DMAs move bytes, but the
        # BIR verifier requires every producer of an f32r-matmul operand
        # to declare an f32r output.
        return ap.bitcast(F32R) if flag else ap

    def rr(ap, site):
        # fp32 matmul = 4 cycles/col on PE; f32r (fp32 bits, 11-bit
        # mantissa read) = 1 cycle/col for moving>=256. The model is
        # chaotic (any per-op rounding amplifies ~300-500x into h), so
        # f32r is enabled only for sites measured insensitive in the
        # numpy rounding study; everything else stays exact fp32.
        return ap.bitcast(F32R) if site in f32r_sites else ap

    def vap(sliced, dims):
        # dims[0] is the partition dim with its step given in PARTITIONS;
        # bass APs use flat element addressing, so scale by the tensor's
        # partition pitch (taken from the sliced AP's own partition step).
        pitch = sliced.ap[0][0]
        d0 = [dims[0][0] * pitch, dims[0][1]]
        return bass.AP(
            tensor=sliced.tensor,
            offset=sliced.offset,
            ap=[d0] + [list(d) for d in dims[1:]],
        )

    def esc_scatter(dst_esc, src_row_of_mol, m):
        """DMA a [1, 812] molecule row into esc rows 16m..16m+16."""
        nc.sync.dma_start(
            vap(dst_esc[16 * m : 16 * m + ESC_Q, :], [[1, ESC_Q], [1, ESC_F]]),
            vap(src_row_of_mol[:, 0:765], [[1, 1], [ESC_F, ESC_Q], [1, ESC_F]]),
        )
        nc.sync.dma_start(
            vap(dst_esc[16 * m + ESC_Q : 16 * m + 16, 0:ESC_R], [[1, 1], [1, ESC_R]]),
            vap(src_row_of_mol[:, 765:E], [[1, 1], [1, ESC_R]]),
        )

    def esc_gather(dst_row_of_mol, src_esc, m):
        """DMA esc rows 16m..16m+16 back into a [1, 812] molecule row."""
        nc.sync.dma_start(
            vap(dst_row_of_mol[:, 0:765], [[1, 1], [ESC_F, ESC_Q], [1, ESC_F]]),
            vap(src_esc[16 * m : 16 * m + ESC_Q, :], [[1, ESC_Q], [1, ESC_F]]),
        )
        nc.sync.dma_start(
            vap(dst_row_of_mol[:, 765:E], [[1, 1], [1, ESC_R]]),
            vap(src_esc[16 * m + ESC_Q : 16 * m + 16, 0:ESC_R], [[1, 1], [1, ESC_R]]),
        )

    with tile.TileContext(nc) as tc, ExitStack() as ctx:
        pers = ctx.enter_context(tc.tile_pool(name="pers", bufs=1))
        wpool = ctx.enter_context(tc.tile_pool(name="wpool", bufs=2))
        st1 = ctx.enter_context(tc.tile_pool(name="st1", bufs=1))
        st2 = ctx.enter_context(tc.tile_pool(name="st2", bufs=2))
        geo = ctx.enter_context(tc.tile_pool(name="geo", bufs=1))
        mols = ctx.enter_context(tc.tile_pool(name="mols", bufs=2))
        pbig = ctx.enter_context(tc.tile_pool(name="pbig", bufs=2, space="PSUM"))
        psa = ctx.enter_context(tc.tile_pool(name="psa", bufs=2, space="PSUM"))
        ph8 = ctx.enter_context(tc.tile_pool(name="ph8", bufs=1, space="PSUM"))

        # ---- persistent constants / state ----
        any_pre1 = any(
            f"{l}:x1" in f32r_sites or f"{l}:e1" in f32r_sites for l in range(nlayers)
        )
        any_geos = any(f"geo_s:{l}" in f32r_sites for l in range(nlayers))
        # rjsa: rows 0..28 = RJ repeated per molecule block, rows 29..30 =
        # per-edge [s; a]. One K=31 matmul then computes the j-gather AND
        # the wcd @ [s; a] term of pre1 in a single instruction.
        rjsa = pers.tile([N + 2, E * NM], F32)
        for m in range(NM):
            nc.gpsimd.dma_start(
                lab(rjsa[0:N, m * E : (m + 1) * E], any_pre1),
                lab(rj_d[:], any_pre1),
            )
        obs_t = pers.tile([3 * NM, NM], F32)
        nc.gpsimd.dma_start(lab(obs_t[:], any_geos), lab(obs_d[:], any_geos))
        obw_t = pers.tile([NM, 3 * NM], F32)
        nc.gpsimd.dma_start(obw_t[:], obw_d[:])
        win_t = pers.tile([FA + 1, NTP], F32)
        nc.gpsimd.dma_start(win_t[:], win_d[:])
        wout_t = pers.tile([128, 2, FA + 1], F32)
        wr = "wout" in f32r_sites
        nc.gpsimd.dma_start(lab(wout_t[:], wr), lab(wout_d[:], wr))
        hT0_t = pers.tile([FA + 1, NTP], F32)
        nc.gpsimd.dma_start(hT0_t[:], hT0_d[:])
        x24i = pers.tile([3 * NM, N], F32)
        nc.gpsimd.dma_start(x24i[:], x24_d[:])
        ones1 = pers.tile([1, 128], F32)
        nc.vector.memset(ones1[:], 1.0)
        # per-partition constants for the Pool-engine Newton chains
        # (Pool has no tensor_scalar, so constants come in as stride-0
        # broadcast tensor operands): cols 0..3 = 1.5, 0.5, 2.0, 1.0
        cgeo = pers.tile([NM, 4], F32)
        for ci, cv in enumerate((1.5, 0.5, 2.0, 1.0)):
            nc.vector.memset(cgeo[:, ci : ci + 1], cv)

        def cb(ci):
            return vap(cgeo[:, ci : ci + 1], [[1, NM], [0, E]])

        sa2 = rjsa[N : N + 2, :]               # row0 = s (=d^2), row1 = a
        hf = pers.tile([128, 2, NTP], F32)     # h state fp32
        hb = pers.tile([128, 2, NTP], F32)     # matmul operand copy (padded)
        hbr = pers.tile([128, 2, NTP], F32)    # f32r-rounded operand copy

        def hb_for(site):
            return hbr if site in f32r_sites else hb
        x24 = pers.tile([3 * NM, N], F32)
        x24d = pers.tile([3 * NM, 2 * N], F32)

        # h0 = win_w.T @ [h_in; t]
        for mc in range(2):
            ps = psa.tile([128, NTP], F32, tag="psa")
            nc.tensor.matmul(
                ps[:], rr(win_t[:, ts(mc, 128)], "win"), rr(hT0_t[:], "win"),
                start=True, stop=True,
            )
            nc.vector.tensor_copy(hf[:, mc, :], ps[:])
            nc.vector.tensor_copy(hb[:, mc, :], ps[:])
        nc.vector.tensor_copy(x24[:], x24i[:])

        dbg_t = {}

        def dump(name, tile_ap):
            if not dbg:
                return
            if name not in dbg_t:
                dbg_t[name] = nc.dram_tensor(
                    "dbg_" + name, list(tile_ap.shape), F32, kind="ExternalOutput"
                ).ap()
            nc.sync.dma_start(dbg_t[name][:], tile_ap)

        for l in range(nlayers):
            # ---- stream layer weights (double-buffered) ----
            r_pre = f"{l}:x1" in f32r_sites or f"{l}:e1" in f32r_sites
            r_w2 = f"{l}:x2" in f32r_sites or f"{l}:e2" in f32r_sites
            w1i = wpool.tile([128, 2, 2, H], F32, tag="w1i")
            nc.gpsimd.dma_start(lab(w1i[:], r_pre), lab(w1i_d[l], r_pre))
            w1j = wpool.tile([128, 2, 2, H], F32, tag="w1j")
            nc.gpsimd.dma_start(lab(w1j[:], r_pre), lab(w1j_d[l], r_pre))
            w2 = wpool.tile([128, 2, 2, H], F32, tag="w2")
            nc.gpsimd.dma_start(lab(w2[:], r_w2), lab(w2_d[l], r_w2))
            w38 = wpool.tile([128, 2, NM, NM], F32, tag="w38")
            r_x3 = f"{l}:x3" in f32r_sites
            nc.gpsimd.dma_start(lab(w38[:], r_x3), lab(w38_d[l], r_x3))
            attb = wpool.tile([128, 2, 128], F32, tag="attb")
            r_att = f"{l}:att" in f32r_sites
            nc.gpsimd.dma_start(lab(attb[:], r_att), lab(attb_d[l], r_att))
            hw1 = wpool.tile([128, 4, H], F32, tag="hw1")
            r_h1 = f"{l}:h1" in f32r_sites
            nc.gpsimd.dma_start(lab(hw1[:], r_h1), lab(hw1_d[l], r_h1))
            hw2 = wpool.tile([128, 2, H], F32, tag="hw2")
            r_h2 = f"{l}:h2" in f32r_sites
            nc.gpsimd.dma_start(lab(hw2[:], r_h2), lab(hw2_d[l], r_h2))
            wcd = wpool.tile([2, 2, H], F32, tag="wcd")
            nc.gpsimd.dma_start(lab(wcd[:], r_pre), lab(wcd_d[l], r_pre))

            # ============ geometry part 1: diff / dsq (DVE) ============
            nc.vector.tensor_copy(x24d[:, 0:N], x24[:])
            nc.vector.tensor_copy(x24d[:, N : 2 * N], x24[:])
            diff = st2.tile([3 * NM, E], F32, tag="diff", bufs=1)
            nc.vector.tensor_tensor(
                vap(diff[:], [[1, 3 * NM], [N, M28], [1, N]]),
                vap(x24[:], [[1, 3 * NM], [0, M28], [1, N]]),
                vap(x24d[:, 1:], [[1, 3 * NM], [1, M28], [1, N]]),
                ALU.subtract,
            )
            if l == 0:
                dump("diff", diff[:])
            dsq = st2.tile([3 * NM, E], F32, tag="dsq", bufs=1)
            nc.vector.tensor_tensor(
                rr(dsq[:], f"geo_s:{l}"), diff[:], diff[:], ALU.mult
            )

            # ================= A-stage =================
            ai = st2.tile([128, 4, NTP], F32, tag="ai", bufs=1)
            for br in range(2):
                s1 = f"{l}:x1" if br == 0 else f"{l}:e1"
                for mc in range(2):
                    ps = psa.tile([128, NTP], F32, tag="psa")
                    for kc in range(2):
                        nc.tensor.matmul(
                            ps[:],
                            rr(w1i[:, br, kc, ts(mc, 128)], s1),
                            rr(hb_for(s1)[:, kc, :], s1),
                            start=(kc == 0),
                            stop=(kc == 1),
                        )
                    nc.vector.tensor_copy(ai[:, 2 * br + mc, :], ps[:])
            if l == 0:
                dump("ai", ai[:])
            # ajt rows 0..28: h_j @ w1j per node; rows 29..30: wcd, so one
            # K=31 matmul against rjsa computes j-gather + wcd @ [s; a]
            ajt = st1.tile([N + 2, 2, NM, H], F32, tag="ajt")
            for br in range(2):
                s1 = f"{l}:x1" if br == 0 else f"{l}:e1"
                # wcd -> ajt rows 29..30 for every molecule slot (label-only
                # f32r DMA; rounding happens at the matmul read)
                wsl = wcd[:, br, :]
                dsl = ajt[N : N + 2, br, 0, :]
                nc.gpsimd.dma_start(
                    lab(
                        bass.AP(
                            tensor=dsl.tensor,
                            offset=dsl.offset,
                            ap=[list(dsl.ap[0]), [H, NM], [1, H]],
                        ),
                        r_pre,
                    ),
                    lab(
                        bass.AP(
                            tensor=wsl.tensor,
                            offset=wsl.offset,
                            ap=[list(wsl.ap[0]), [0, NM], [1, H]],
                        ),
                        r_pre,
                    ),
                )
                for nk in range(2):
                    ps = psa.tile([116, NTP], F32, tag="psa")
                    for kc in range(2):
                        nc.tensor.matmul(
                            ps[:, 0:H],
                            rr(hb_for(s1)[:, kc, nk * 116 : nk * 116 + 116], s1),
                            rr(w1j[:, br, kc, :], s1),
                            start=(kc == 0),
                            stop=(kc == 1),
                        )
                    ajsb = st2.tile([116, H], F32, tag="ajsb")
                    nc.vector.tensor_copy(rr(ajsb[:], s1), ps[:, 0:H])
                    for mm in range(4):
                        nc.sync.dma_start(
                            lab(ajt[0:N, br, nk * 4 + mm, :], r_pre),
                            lab(ajsb[29 * mm : 29 * mm + 29, :], r_pre),
                        )

            # ========= geometry part 2: s = per-mol xyz sum (PE) =========
            s8p = ph8.tile([NM, 1024], F32, tag="ph8")
            gs = f"geo_s:{l}"
            nc.tensor.matmul(
                s8p[:, 0:510], rr(obs_t[:], gs), rr(dsq[:, 0:510], gs),
                start=True, stop=True,
            )
            nc.tensor.matmul(
                s8p[:, 512:814], rr(obs_t[:], gs), rr(dsq[:, 510:E], gs),
                start=True, stop=True,
            )
            s8sb = geo.tile([NM, E], F32, tag="s8sb")
            nc.vector.tensor_copy(s8sb[:, 0:510], s8p[:, 0:510])
            nc.vector.tensor_copy(s8sb[:, 510:E], s8p[:, 512:814])
            nc.sync.dma_start(
                lab(vap(sa2[0:1, :], [[1, 1], [E, NM], [1, E]]), any_pre1),
                lab(vap(s8sb[:], [[1, NM], [1, E]]), any_pre1),
            )
            if l == 0:
                dump("s8sb", s8sb[:])

            def emit_newton():
                # d = s * rsqrt(s) (bit-trick seed + 3 Newton iters), then
                # w = 1/(d+1) (fast-reciprocal seed + 3 Newton iters). The
                # serial float chain runs on the otherwise-idle Pool engine:
                # on the DVE it executes as one solid ~20us burst that
                # starves the psv-add -> silu chain feeding the PE. Pool has
                # no tensor_scalar, so the int seeds stay on the DVE (cheap)
                # and iteration constants come from stride-0 broadcasts.
                it8 = geo.tile([NM, E], I32, tag="it8")
                nc.vector.tensor_scalar(
                    it8[:], s8sb[:].bitcast(I32), 1, None,
                    ALU.logical_shift_right
                )
                nc.vector.tensor_scalar(it8[:], it8[:], -1, None,
                                        ALU.bitwise_xor)
                nc.vector.tensor_scalar(it8[:], it8[:], MAGIC + 1, None,
                                        ALU.add)
                r8 = it8[:].bitcast(F32)
                sh8 = geo.tile([NM, E], F32, tag="sh8")
                nc.gpsimd.tensor_tensor(sh8[:], s8sb[:], cb(1), ALU.mult)
                t8 = geo.tile([NM, E], F32, tag="t8")
                for _ in range(3):
                    nc.gpsimd.tensor_tensor(t8[:], r8, r8, ALU.mult)
                    nc.gpsimd.tensor_tensor(t8[:], t8[:], sh8[:], ALU.mult)
                    nc.gpsimd.tensor_tensor(t8[:], cb(0), t8[:],
                                            ALU.subtract)
                    nc.gpsimd.tensor_tensor(r8, r8, t8[:], ALU.mult)
                d8 = geo.tile([NM, E], F32, tag="d8")
                nc.gpsimd.tensor_tensor(d8[:], s8sb[:], r8, ALU.mult)
                if l == 0:
                    nc.sync.dma_start(
                        lab(vap(sa2[1:2, :], [[1, 1], [E, NM], [1, E]]),
                            any_pre1),
                        lab(vap(d8[:], [[1, NM], [1, E]]), any_pre1),
                    )
                # w = 1/(1+d): y0 = bits(0x7EF311C3 - bits(1+d)), then
                # y <- y*(2 - (1+d)*y) three times (quadratic convergence
                # from the ~5e-2 seed error down to fp32 exact).
                nc.gpsimd.tensor_tensor(t8[:], d8[:], cb(3), ALU.add)
                iw8 = geo.tile([NM, E], I32, tag="iw8")
                nc.vector.tensor_scalar(iw8[:], t8[:].bitcast(I32), -1, None,
                                        ALU.bitwise_xor)
                nc.vector.tensor_scalar(iw8[:], iw8[:], 0x7EF311C4, None,
                                        ALU.add)
                w8 = iw8[:].bitcast(F32)
                u8 = geo.tile([NM, E], F32, tag="u8")
                for _ in range(3):
                    nc.gpsimd.tensor_tensor(u8[:], t8[:], w8, ALU.mult)
                    nc.gpsimd.tensor_tensor(u8[:], cb(2), u8[:],
                                            ALU.subtract)
                    nc.gpsimd.tensor_tensor(w8, w8, u8[:], ALU.mult)
                if l == 0:
                    dump("sa2", sa2[:])
                    dump("w8", w8)
                return w8

            w8 = emit_newton()

            # ============ branch MLPs (e: br=1 first, then x: br=0) ============
            # Stage-major with lag interleave: the PE stream alternates
            # pre1[m] / w2[m-2] / head[m-3] so every matmul's inputs (silu
            # outputs) were produced while the PE ran other molecules.
            # The e-branch gating (em mult + segment reduce) runs on the
            # Pool engine so the DVE never head-of-line-blocks the silu
            # chain that feeds the PE.
            if l == 0:
                dump("ajt", ajt[:])
            agg = st2.tile([128, 2, NTP], F32, tag="agg", bufs=1)
            nc.vector.memset(agg[:, :, NT:NTP], 0.0)
            php = ph8.tile([NM, 1024], F32, tag="ph8")
            sil1_t = {}
            sil2_t = {}

            def pre1_op(br, m):
                s1 = f"{l}:x1" if br == 0 else f"{l}:e1"
                s2 = f"{l}:x2" if br == 0 else f"{l}:e2"
                sil1 = mols.tile([128, 2, E], F32, tag="sil1", bufs=3)
                sil1_t[(br, m)] = sil1
                for mc in range(2):
                    ps = pbig.tile([128, 1024], F32, tag="pp")
                    for col in range(2):
                        po = ps[:, col * 512 : col * 512 + 406]
                        c0 = m * E + col * 406
                        nc.tensor.matmul(
                            po,
                            rr(ajt[:, br, m, ts(mc, 128)], s1),
                            rr(rjsa[:, c0 : c0 + 406], s1),
                            start=True,
                            stop=True,
                        )
                    psv = vap(ps[:], [[1, 128], [512, 2], [1, 406]])
                    nc.vector.tensor_tensor(
                        psv,
                        vap(
                            ai[:, 2 * br + mc, m * N : (m + 1) * N],
                            [[1, 128], [0, M28], [1, N]],
                        ),
                        psv,
                        ALU.add,
                    )
                    nc.scalar.activation(
                        rr(vap(sil1[:, mc, :], [[1, 128], [406, 2], [1, 406]]),
                           s2),
                        psv,
                        ACTF.Silu,
                    )
                if l == 0 and m == 0:
                    dump(f"sil1_{br}", sil1[:])

            def w2_op(br, m):
                s2 = f"{l}:x2" if br == 0 else f"{l}:e2"
                shead = f"{l}:x3" if br == 0 else f"{l}:att"
                sil1 = sil1_t.pop((br, m))
                sil2 = mols.tile([128, 2, E], F32, tag="sil2", bufs=2)
                sil2_t[(br, m)] = sil2
                for mc in range(2):
                    ps = pbig.tile([128, 1024], F32, tag="pp")
                    for col in range(2):
                        po = ps[:, col * 512 : col * 512 + 406]
                        for kc in range(2):
                            nc.tensor.matmul(
                                po,
                                rr(w2[:, br, kc, ts(mc, 128)], s2),
                                rr(sil1[:, kc, col * 406 : col * 406 + 406], s2),
                                start=(kc == 0),
                                stop=(kc == 1),
                            )
                    nc.scalar.activation(
                        rr(vap(sil2[:, mc, :], [[1, 128], [406, 2], [1, 406]]),
                           shead),
                        vap(ps[:], [[1, 128], [512, 2], [1, 406]]),
                        ACTF.Silu,
                    )
                if l == 0 and m == 0:
                    dump(f"sil2_{br}", sil2[:])

            def head_op(br, m):
                sil2 = sil2_t.pop((br, m))
                if br == 0:
                    for col in range(2):
                        for kc in range(2):
                            nc.tensor.matmul(
                                php[:, col * 512 : col * 512 + 406],
                                rr(w38[:, kc, m, :], f"{l}:x3"),
                                rr(sil2[:, kc, col * 406 : col * 406 + 406],
                                   f"{l}:x3"),
                                start=(m == 0 and kc == 0),
                                stop=(m == NM - 1 and kc == 1),
                                skip_group_check=True,
                            )
                else:
                    atp = pbig.tile([128, 1024], F32, tag="pp")
                    for col in range(2):
                        for kc in range(2):
                            nc.tensor.matmul(
                                atp[:, col * 512 : col * 512 + 406],
                                rr(attb[:, kc, :], f"{l}:att"),
                                rr(sil2[:, kc, col * 406 : col * 406 + 406],
                                   f"{l}:att"),
                                start=(kc == 0),
                                stop=(kc == 1),
                            )
                    eg_sb = mols.tile([128, E], F32, tag="eg_sb")
                    nc.scalar.activation(
                        vap(eg_sb[:], [[1, 128], [406, 2], [1, 406]]),
                        vap(atp[:], [[1, 128], [512, 2], [1, 406]]),
                        ACTF.Tanh,
                        scale=0.5,
                    )
                    nc.vector.tensor_scalar(
                        eg_sb[:], eg_sb[:], 0.5, 0.5, ALU.mult, ALU.add
                    )
                    if l == 0 and m == 0:
                        dump("eg_sb", eg_sb[:])
                    em = mols.tile([128, 2, E], F32, tag="em", bufs=2)
                    for mc in range(2):
                        nc.vector.tensor_tensor(
                            em[:, mc, :], sil2[:, mc, :], eg_sb[:], ALU.mult
                        )
                        # segment-sum of the 28 m'-blocks via an in-place
                        # contiguous add tree (Pool can't do free-axis
                        # reduce); 28*29 = 812 → 406 → 203 → 87 → 29
                        ev = em[:, mc, :]
                        for lo, mid, w in (
                            (0, 406, 406),   # 28 -> 14 blocks
                            (0, 203, 203),   # 14 -> 7
                            (0, 87, 87),     # 7 -> 3 (+1 leftover at 174)
                            (0, 29, 29),     # 3 -> 1 (+1 leftover at 58)
                            (0, 58, 29),
                        ):
                            nc.vector.tensor_tensor(
                                ev[:, lo : lo + w], ev[:, lo : lo + w],
                                ev[:, mid : mid + w], ALU.add,
                            )
                        nc.vector.tensor_tensor(
                            agg[:, mc, m * N : (m + 1) * N],
                            ev[:, 0:N], ev[:, 174 : 174 + N], ALU.add,
                        )

            for br in (0, 1):
                for step in range(NM + 3):
                    if step < NM:
                        pre1_op(br, step)
                    if 0 <= step - 2 < NM:
                        w2_op(br, step - 2)
                    if 0 <= step - 3 < NM:
                        head_op(br, step - 3)

            if True:
                if True:
                    # ---- x tail ----
                    phi8 = geo.tile([NM, E], F32, tag="phi8")
                    nc.vector.tensor_copy(
                        phi8[:],
                        vap(php[:], [[1, NM], [512, 2], [1, 406]]),
                    )
                    if l == 0:
                        dump("phi8", phi8[:])
                    g8 = geo.tile([NM, E], F32, tag="g8")
                    nc.scalar.activation(g8[:], phi8[:], ACTF.Tanh)
                    wg8 = geo.tile([NM, E], F32, tag="wg8")
                    nc.vector.tensor_tensor(wg8[:], w8, g8[:], ALU.mult)
                    wg24 = pbig.tile([3 * NM, 1024], F32, tag="pp")
                    gu = f"geo_u:{l}"
                    nc.tensor.matmul(
                        wg24[:, 0:406],
                        rr(obw_t[:], gu),
                        rr(wg8[:, 0:406], gu),
                        start=True,
                        stop=True,
                    )
                    nc.tensor.matmul(
                        wg24[:, 512:918],
                        rr(obw_t[:], gu),
                        rr(wg8[:, 406:E], gu),
                        start=True,
                        stop=True,
                    )
                    u_vec = st2.tile([3 * NM, E], F32, tag="u_vec", bufs=1)
                    nc.vector.tensor_tensor(
                        vap(u_vec[:], [[1, 3 * NM], [406, 2], [1, 406]]),
                        vap(diff[:], [[1, 3 * NM], [406, 2], [1, 406]]),
                        vap(wg24[:], [[1, 3 * NM], [512, 2], [1, 406]]),
                        ALU.mult,
                    )
                    if l == 0:
                        dump("wg8", wg8[:])
                        dump("u_vec", u_vec[:])
                    xinc = st2.tile([3 * NM, N], F32, tag="xinc")
                    nc.vector.tensor_reduce(
                        xinc[:],
                        vap(u_vec[:], [[1, 3 * NM], [1, N], [N, M28]]),
                        axis=mybir.AxisListType.X,
                        op=ALU.add,
                    )
                    x24n = pers.tile([3 * NM, N], F32, tag=f"x24n_{l % 2}")
                    nc.vector.tensor_tensor(x24n[:], x24[:], xinc[:], ALU.add)
                    x24 = x24n

            if l == 0:
                dump("agg", agg[:])
            # ================= h update =================
            hm1 = st2.tile([128, 2, NTP], F32, tag="hm1", bufs=1)
            rhs_list = [hb[:, 0, :], hb[:, 1, :], agg[:, 0, :], agg[:, 1, :]]
            for mc in range(2):
                ps = psa.tile([128, NTP], F32, tag="psa")
                for kc in range(4):
                    nc.tensor.matmul(
                        ps[:],
                        rr(hw1[:, kc, ts(mc, 128)], f"{l}:h1"),
                        rr(rhs_list[kc], f"{l}:h1"),
                        start=(kc == 0),
                        stop=(kc == 3),
                    )
                nc.scalar.activation(
                    rr(hm1[:, mc, :], f"{l}:h2"), ps[:], ACTF.Silu
                )
            hfn = pers.tile([128, 2, NTP], F32, tag=f"hf_{l % 2}")
            for mc in range(2):
                ps = psa.tile([128, NTP], F32, tag="psa")
                for kc in range(2):
                    nc.tensor.matmul(
                        ps[:],
                        rr(hw2[:, kc, ts(mc, 128)], f"{l}:h2"),
                        rr(hm1[:, kc, :], f"{l}:h2"),
                        start=(kc == 0),
                        stop=(kc == 1),
                    )
                nc.vector.tensor_tensor(hfn[:, mc, :], hf[:, mc, :], ps[:], ALU.add)
                nc.vector.tensor_copy(hb[:, mc, :], hfn[:, mc, :])
            hf = hfn
            if l == 0:
                dump("hf1", hf[:])
                dump("x24_1", x24[:])

        # ================= output =================
        xd = st2.tile([3 * NM, N], F32, tag="xd")
        nc.vector.tensor_tensor(xd[:], x24[:], x24i[:], ALU.subtract)
        mean = st2.tile([3 * NM, 1], F32, tag="mean")
        nc.vector.tensor_reduce(
            mean[:], xd[:], axis=mybir.AxisListType.X, op=ALU.add
        )
        nc.vector.tensor_scalar(mean[:], mean[:], 1.0 / N, None, ALU.mult)
        xo = st2.tile([3 * NM, N], F32, tag="xo")
        nc.vector.tensor_scalar(xo[:], xd[:], mean[:], None, ALU.subtract)
        out8 = st1.tile([FA + 3, NT], F32, tag="out8")
        for m in range(NM):
            nc.sync.dma_start(
                out8[0:3, m * N : (m + 1) * N], xo[3 * m : 3 * m + 3, :]
            )
        ps = psa.tile([FA + 1, NTP], F32, tag="psa")
        for kc in range(2):
            nc.tensor.matmul(
                ps[:],
                rr(wout_t[:, kc, :], "wout"),
                rr(hb_for("wout")[:, kc, :], "wout"),
                start=(kc == 0),
                stop=(kc == 1),
            )
        hosb = st2.tile([FA + 1, NTP], F32, tag="hosb")
        nc.vector.tensor_copy(hosb[:], ps[:])
        nc.sync.dma_start(out8[3 : 3 + FA, :], hosb[0:FA, 0:NT])
        nc.sync.dma_start(out_d[:], out8[:])

    nc.compile()
    _BUILD_CACHE[key] = nc
    return nc


# --------------------------------------------------------------------------
# host side
# --------------------------------------------------------------------------
def _is_canonical(inputs):
    ei = np.asarray(inputs["edge_indices"])
    if not np.all(ei == _canonical_edge_indices()[None]):
        return False
    if not np.all(np.asarray(inputs["node_mask"]) == 1.0):
        return False
    if not np.all(np.asarray(inputs["edge_mask"]) == 1.0):
        return False
    for k in ("win_b", "wout_b", "e_b1", "e_b2", "att_b", "h_b1", "h_b2",
              "x_b1", "x_b2"):
        if not np.all(np.asarray(inputs[k]) == 0.0):
            return False
    return True


def _prep_shared(inputs):
    f = np.float32
    x_w1 = np.asarray(inputs["x_w1"], f)
    e_w1 = np.asarray(inputs["e_w1"], f)
    x_w2 = np.asarray(inputs["x_w2"], f)
    e_w2 = np.asarray(inputs["e_w2"], f)
    x_w3 = np.asarray(inputs["x_w3"], f)
    att_w = np.asarray(inputs["att_w"], f)
    h_w1 = np.asarray(inputs["h_w1"], f)
    h_w2 = np.asarray(inputs["h_w2"], f)

    W1I = np.zeros((L, 128, 2, 2, H), f)
    W1J = np.zeros((L, 128, 2, 2, H), f)
    W2 = np.zeros((L, 128, 2, 2, H), f)
    W38 = np.zeros((L, 128, 2, NM, NM), f)
    ATTB = np.zeros((L, 128, 2, 128), f)
    HW1 = np.zeros((L, 128, 4, H), f)
    HW2 = np.zeros((L, 128, 2, H), f)
    WCD = np.zeros((L, 2, 2, H), f)
    for l in range(L):
        for br, w1 in enumerate((x_w1[l], e_w1[l])):
            for kc in range(2):
                W1I[l, :, br, kc, :] = w1[kc * 128 : (kc + 1) * 128, :]
                W1J[l, :, br, kc, :] = w1[H + kc * 128 : H + (kc + 1) * 128, :]
            WCD[l, 0, br, :] = w1[2 * H, :]
            WCD[l, 1, br, :] = w1[2 * H + 1, :]
        for br, w2 in enumerate((x_w2[l], e_w2[l])):
            for kc in range(2):
                W2[l, :, br, kc, :] = w2[kc * 128 : (kc + 1) * 128, :]
        for kc in range(2):
            for m in range(NM):
                W38[l, :, kc, m, m] = x_w3[l][kc * 128 : (kc + 1) * 128, 0]
            ATTB[l, :, kc, :] = att_w[l][kc * 128 : (kc + 1) * 128, 0:1]
            HW2[l, :, kc, :] = h_w2[l][kc * 128 : (kc + 1) * 128, :]
        for kc in range(4):
            HW1[l, :, kc, :] = h_w1[l][kc * 128 : (kc + 1) * 128, :]

    WIN = np.zeros((FA + 1, NTP), f)
    WIN[:, :H] = np.asarray(inputs["win_w"], f)
    WOUT = np.zeros((128, 2, FA + 1), f)
    wout = np.asarray(inputs["wout_w"], f)
    for kc in range(2):
        WOUT[:, kc, :] = wout[kc * 128 : (kc + 1) * 128, :]
    RJ = _rj_matrix()
    OBS = np.kron(np.eye(NM, dtype=f), np.ones((3, 1), f))
    OBW = (SCALE * np.kron(np.eye(NM, dtype=f), np.ones((1, 3), f))).astype(f)
    return dict(
        W1I=W1I, W1J=W1J, W2=W2, W38=W38, ATTB=ATTB, HW1=HW1, HW2=HW2,
        WCD=WCD, WIN=WIN, WOUT=WOUT, RJ=RJ, OBS=OBS, OBW=OBW,
    )


def _prep_core(inputs, c):
    f = np.float32
    sl = slice(c * NM, (c + 1) * NM)
    h_in = np.asarray(inputs["h_in"], f)[sl]
    t = np.asarray(inputs["t"], f)[sl]
    x_in = np.asarray(inputs["x_in"], f)[sl]
    hT0 = np.zeros((FA + 1, NTP), f)
    hT0[:, :NT] = (
        np.concatenate([h_in, t], -1).reshape(NT, FA + 1).T
    )
    x24 = np.ascontiguousarray(x_in.transpose(0, 2, 1).reshape(3 * NM, N))
    return {"hT0": hT0, "x24": x24}


def kernel(**inputs):
    if not _is_canonical(inputs):
        p = {k: np.asarray(v, np.float32) for k, v in inputs.items()
             if k not in ("edge_indices",)}
        return _numpy_forward(
            np.asarray(inputs["x_in"], np.float32),
            np.asarray(inputs["h_in"], np.float32),
            np.asarray(inputs["t"], np.float32),
            np.asarray(inputs["node_mask"], np.float32),
            np.asarray(inputs["edge_mask"], np.float32),
            np.asarray(inputs["edge_indices"]),
            p,
        )

    from concourse.bass_utils import run_bass_kernel_spmd

    nc = _build(f32r_sites=F32R_SITES)
    _LAST_EXEC_NS.clear()
    shared = _prep_shared(inputs)
    in_maps = []
    for c in range(NC):
        m = dict(shared)
        m.update(_prep_core(inputs, c))
        in_maps.append(m)
    res = run_bass_kernel_spmd(nc, in_maps, core_ids=list(range(NC)))
    _LAST_RES["res"] = res
    if res.exec_time_ns is not None:
        _LAST_EXEC_NS["ns"] = res.exec_time_ns
    out = np.zeros((B, N, FA + 3), np.float32)
    for c in range(NC):
        o = res.results[c]["out"]  # [8, 232]
        out[c * NM : (c + 1) * NM] = o.reshape(FA + 3, NM, N).transpose(1, 2, 0)
    return out

